# revision 4
# baseline (speedup 1.0000x reference)
"""GCN (2x GCNConv + mean-pool + linear) on 8 Trainium2 NeuronCores.

Single-launch fused kernel.  The per-launch dispatch overhead through the
PJRT tunnel (~86 ms fixed + ~0.1 ms/MB shipped) dominates the on-device
execution (~1 ms/layer), so v2 fuses both GCN layers, the pooling and the
final linear into ONE launch and strips the shipped inputs down to compact
per-core edge metadata (~1.2 MB/core):

  - x shard (own nodes, dinsq-prescaled, fp16) -> on-device AllGather ->
    on-device build of the [N, 128] layer-1 gather table (cols 4+ garbage,
    never read: matmuls only consume 0:4).
  - layer 1 runs the baseline message-passing scheme (SWDGE row gather +
    is_equal one-hot + PE scatter-add in PSUM), writes the prescaled h1
    shard; an AllGather produces each core's full [N, 128] fp16 layer-2
    table on device (no host round-trip, no replicated upload).
  - layer 2 ditto + mean-pool matmul + final linear; host sums the 8
    per-core [G, OUT] partials (pooling weights are built on device from
    per-node graph ids via is_equal×mult).
  - iota / identity matrices are built on device; gather indices ship as a
    single 16-partition wrap and are replicated to 128 partitions on device.

Normalization algebra (unchanged from baseline): with deg including the
self loop, the stored table is t_i = dinsq_i * relu(P_i) * dinsq_i where
P_i = W^T (sum_{e->i} t_src + t_i_own) + b*sqrtdeg_i, so the src-side
dinsq is pre-folded into the table and the dst-side dinsq commutes out of
the relu; the pooling weights carry the final dst dinsq * 1/cnt.
"""

import sys
from contextlib import ExitStack

for _p in ("/opt/trn_rl_repo",):
    if _p not in sys.path:
        sys.path.insert(0, _p)

import numpy as np

import concourse.bass as bass
import concourse.mybir as mybir
import concourse.tile as tile
from concourse import bacc
from concourse.bass_utils import run_bass_kernel_spmd
from concourse.library_config import mlp

FP16 = mybir.dt.float16
F32 = mybir.dt.float32
I16 = mybir.dt.int16
FP16_NP = np.float16


class Cfg:
    def __init__(self, N=100000, E=1600000, G=100, DIN=3, H=128, OUT=10,
                 NCORES=8, WT=8, SC_SIZE=25088):
        self.N, self.E, self.G = N, E, G
        self.DIN, self.H, self.OUT = DIN, H, OUT
        self.NCORES = NCORES
        assert N % NCORES == 0
        self.NPC = N // NCORES                      # nodes per core
        self.NT = (self.NPC + 127) // 128           # dst tiles per core
        self.LAST_VALID = self.NPC - (self.NT - 1) * 128
        self.WT = WT                                # tiles per wave
        self.NW = (self.NT + WT - 1) // WT
        assert SC_SIZE <= 32768
        self.SC_SIZE = SC_SIZE                      # src chunk rows (int16 idx)
        self.NSC = (N + SC_SIZE - 1) // SC_SIZE


FULL = Cfg()


# --------------------------------------------------------------------------
# host preprocessing
# --------------------------------------------------------------------------

def preprocess(cfg, x, edge_index, batch):
    N, G, NC = cfg.N, cfg.G, cfg.NCORES
    NPC, NT, NSC, WT = cfg.NPC, cfg.NT, cfg.NSC, cfg.WT
    src = np.asarray(edge_index[0], dtype=np.int64)
    dst = np.asarray(edge_index[1], dtype=np.int64)
    batch = np.asarray(batch, dtype=np.int64)
    x = np.asarray(x, dtype=np.float32)

    deg = (np.bincount(dst, minlength=N) + 1.0).astype(np.float32)
    dinsq = (1.0 / np.sqrt(deg)).astype(np.float32)
    sqrtdeg = np.sqrt(deg).astype(np.float32)
    cnt = np.bincount(batch, minlength=G).astype(np.float32)
    invcnt = (1.0 / np.maximum(cnt, 1.0)).astype(np.float32)

    # prescaled node features, padded to 4 cols, node-major fp16
    xs = np.zeros((N, 4), dtype=FP16_NP)
    xs[:, :cfg.DIN] = (x * dinsq[:, None]).astype(FP16_NP)

    core = dst // NPC
    dst_local = dst - core * NPC
    tl = dst_local >> 7
    sc = src // cfg.SC_SIZE
    key = (core * NT + tl) * NSC + sc
    order = np.argsort(key, kind="stable")
    key_s = key[order]
    src_s = src[order]
    sc_s = sc[order]

    counts = np.bincount(key_s, minlength=NC * NT * NSC).reshape(NC, NT, NSC)
    # per-(tile, srcchunk) slot count, padded to 32 and uniform across cores
    GRAIN = 32
    P = ((counts.max(axis=0) + GRAIN - 1) // GRAIN * GRAIN).astype(np.int64)

    waves = [list(range(w * WT, min((w + 1) * WT, NT))) for w in range(cfg.NW)]
    # group = (wave, srcchunk): tiles' slot ranges concatenated, chunked by 128
    slot_base = np.zeros((NT, NSC), dtype=np.int64)   # global slot index
    gmeta = []     # per wave: per s: (idx_col0, nidx, msgcol0, nch)
    wmms = []      # per wave: ordered list of (mcol, j_in_wave, width, gcol)
    pos = 0        # global chunk counter
    SENT = 16384.0
    for w, wtiles in enumerate(waves):
        wmeta = []
        wave_chunk0 = pos
        mms = []
        for s in range(NSC):
            c0 = pos
            off = 0     # slot offset within group
            spans = []  # (t, slot_lo, slot_hi) within group
            for t in wtiles:
                slot_base[t, s] = c0 * 128 + off
                if P[t, s]:
                    spans.append((t, off, off + int(P[t, s])))
                off += int(P[t, s])
            nch = (off + 127) // 128
            for k in range(nch):
                lo, hi = k * 128, (k + 1) * 128
                sp = [t for t, a, b in spans if a < hi and b > lo]
                if not sp:
                    continue
                j0, j1 = sp[0] - wtiles[0], sp[-1] - wtiles[0]
                jj = j0
                while jj <= j1:          # split windows at psum-bank groups
                    je = min(j1, (jj // 4) * 4 + 3)
                    mms.append((c0 - wave_chunk0 + k, jj, je - jj + 1, c0 + k))
                    jj = je + 1
            pos += nch
            wmeta.append((c0 * 8, nch * 128, c0 - wave_chunk0, nch))
        gmeta.append(wmeta)
        wmms.append(mms)
    TOTCH = pos
    CW = max(sum(gmeta[w][s][3] for s in range(NSC))
             for w in range(cfg.NW))

    # scatter edge data into padded per-core arrays
    idx_all = np.zeros((NC, TOTCH * 128), dtype=np.int16)
    dstw_all = np.full((NC, TOTCH * 128), SENT, dtype=FP16_NP)
    bstart = np.zeros(NC * NT * NSC, dtype=np.int64)
    cflat = counts.reshape(-1)
    bstart[1:] = np.cumsum(cflat)[:-1]
    rank = np.arange(len(key_s)) - bstart[key_s]
    ccore = key_s // (NT * NSC)
    rem = key_s % (NT * NSC)
    dest = slot_base.reshape(-1)[rem] + rank
    idx_all[ccore, dest] = (src_s - sc_s * cfg.SC_SIZE).astype(np.int16)
    # dst index relative to the wave's first tile
    wavebase = (tl[order] // WT) * WT * 128
    dstw_all[ccore, dest] = (dst_local[order] - wavebase).astype(FP16_NP)

    # single 16-partition wrap (replicated to 128 partitions on device)
    idx16 = np.ascontiguousarray(
        idx_all.reshape(NC, TOTCH * 8, 16).transpose(0, 2, 1))
    dstw = np.ascontiguousarray(
        dstw_all.reshape(NC, TOTCH, 128).transpose(0, 2, 1))

    # per-core per-node columns (padded to NT*128)
    NPAD = NT * 128
    invdeg_col = np.zeros((NC, 128, NT), dtype=np.float32)
    sqrtdeg_row = np.ones((NC, 1, NPAD), dtype=np.float32)
    bcol = np.full((NC, 128, NT), SENT, dtype=np.float32)  # graph id per node
    wvc = np.zeros((NC, 128, NT), dtype=np.float32)      # dinsq*invcnt
    xss = np.zeros((NC, NPC, 4), dtype=FP16_NP)          # own prescaled x rows
    for c in range(NC):
        idx = np.arange(NPAD) + c * NPC
        valid = np.arange(NPAD) < NPC
        idx = np.where(valid, idx, 0)
        iv = np.where(valid, (dinsq * dinsq)[idx], 1.0).astype(np.float32)
        invdeg_col[c] = iv.reshape(NT, 128).T
        sqrtdeg_row[c, 0] = np.where(valid, sqrtdeg[idx], 1.0)
        bc = np.where(valid, batch[idx].astype(np.float32), SENT)
        bcol[c] = bc.reshape(NT, 128).T
        wv = np.where(valid, dinsq[idx] * invcnt[batch[idx]], 0.0)
        wvc[c] = wv.astype(np.float32).reshape(NT, 128).T
        xss[c] = xs[c * NPC:(c + 1) * NPC]

    return dict(
        xs=xs, xss=xss, idx16=idx16, dstw=dstw,
        invdeg_col=invdeg_col, sqrtdeg_row=sqrtdeg_row,
        bcol=bcol, wvc=wvc,
        waves=waves, gmeta=gmeta, wmms=wmms,
        TOTCH=TOTCH, CW=CW, deg=deg,
    )


# --------------------------------------------------------------------------
# fused kernel builder
# --------------------------------------------------------------------------

def build_fused(cfg, meta, has_b1, has_b2):
    N, G, OUT = cfg.N, cfg.G, cfg.OUT
    NT, NSC, WT, NPC = cfg.NT, cfg.NSC, cfg.WT, cfg.NPC
    TOTCH, CW = meta["TOTCH"], meta["CW"]
    waves, gmeta, wmms = meta["waves"], meta["gmeta"], meta["wmms"]
    NPAD = NT * 128
    NBF = N // 128                   # full 128-row blocks of the node table
    NTAIL = N - NBF * 128
    NC = cfg.NCORES
    GROUPS = [list(range(NC))]

    assert cfg.SC_SIZE % 128 == 0

    nc = bacc.Bacc("TRN2", target_bir_lowering=False, debug=False,
                   num_devices=NC, num_swdge_queues=4,
                   dynamic_dma_scratch_size=32768)
    xsr_d = nc.dram_tensor("xsr", [N, 4], FP16, kind="ExternalInput")
    xss_d = nc.dram_tensor("xss", [NPC, 4], FP16, kind="ExternalInput")
    idx_d = nc.dram_tensor("idx", [16, TOTCH * 8], I16, kind="ExternalInput")
    dstw_d = nc.dram_tensor("dstw", [128, TOTCH], FP16, kind="ExternalInput")
    # scalar operands of is_equal must be f32 on DVE; dstw/bcol are converted
    # (or shipped) as f32 accordingly
    w1_d = nc.dram_tensor("w1", [4, 128], F32, kind="ExternalInput")
    w2_d = nc.dram_tensor("w2", [128, 128], F32, kind="ExternalInput")
    wl_d = nc.dram_tensor("wl", [128, OUT], F32, kind="ExternalInput")
    if has_b1:
        b1_d = nc.dram_tensor("b1r", [1, 128], F32, kind="ExternalInput")
    if has_b2:
        b2_d = nc.dram_tensor("b2r", [1, 128], F32, kind="ExternalInput")
    if has_b1 or has_b2:
        sqd_d = nc.dram_tensor("sqd", [1, NPAD], F32, kind="ExternalInput")
    ivd_d = nc.dram_tensor("ivd", [128, NT], F32, kind="ExternalInput")
    bcol_d = nc.dram_tensor("bcol", [128, NT], F32, kind="ExternalInput")
    wvc_d = nc.dram_tensor("wvc", [128, NT], F32, kind="ExternalInput")
    out_d = nc.dram_tensor("out", [G, OUT], F32, kind="ExternalOutput")

    relu = mybir.ActivationFunctionType.Relu
    copy_fn = mybir.ActivationFunctionType.Copy
    iseq = mybir.AluOpType.is_equal

    with tile.TileContext(nc) as tc:
        nc.gpsimd.load_library(mlp)
        with ExitStack() as ctx:
            const = ctx.enter_context(tc.tile_pool(name="const", bufs=1))
            dram = ctx.enter_context(tc.tile_pool(name="dram", bufs=1,
                                                  space="DRAM"))

            # ---- DRAM internals. The layer-1 gather table is split per
            # src-chunk so wave gathers start as soon as their chunk is built.
            xpad_ts = [dram.tile([min(N, (s + 1) * cfg.SC_SIZE) - s * cfg.SC_SIZE,
                                  128], FP16, name=f"xpad{s}")
                       for s in range(NSC)]
            h1b_t = dram.tile([NPC, 128], FP16)
            h1full_t = dram.tile([N, 128], FP16, addr_space="Shared")

            # ---- constants
            idx_t = const.tile([128, TOTCH * 8], I16)
            for k in range(8):
                nc.sync.dma_start(idx_t[16 * k:16 * k + 16, :], idx_d[:])
            dstw_t = const.tile([128, TOTCH], F32)
            iota16 = const.tile([128, WT * 128], I16)
            nc.gpsimd.iota(iota16[:], [[1, WT * 128]], channel_multiplier=0)
            iota_t = const.tile([128, WT * 128], FP16)
            nc.any.tensor_copy(iota_t[:], iota16[:])
            ig16 = const.tile([128, 128], I16)
            nc.gpsimd.iota(ig16[:], [[1, 128]], channel_multiplier=0)
            pid16 = const.tile([128, 1], I16)
            nc.gpsimd.iota(pid16[:], [[1, 1]], channel_multiplier=1)
            pidf = const.tile([128, 1], F32)
            nc.any.tensor_copy(pidf[:], pid16[:])
            ident_t = const.tile([128, 128], F32)
            nc.vector.tensor_scalar(ident_t[:], ig16[:], pidf[:], None, iseq)
            identb_t = const.tile([128, 128], FP16)
            nc.any.tensor_copy(identb_t[:], ident_t[:])
            iotag_t = const.tile([128, 128], FP16)
            nc.any.tensor_copy(iotag_t[:], ig16[:])
            zc_t = const.tile([1, 512], FP16)
            nc.vector.memset(zc_t[:], 0.0)
            w1_t = const.tile([4, 128], F32)
            nc.sync.dma_start(w1_t[:], w1_d[:])
            w2_t = const.tile([128, 128], F32)
            nc.sync.dma_start(w2_t[:], w2_d[:])
            wl_t = const.tile([128, OUT], F32)
            nc.sync.dma_start(wl_t[:], wl_d[:])
            if has_b1:
                b1r_t = const.tile([1, 128], F32)
                nc.sync.dma_start(b1r_t[:], b1_d[:])
            if has_b2:
                b2r_t = const.tile([1, 128], F32)
                nc.sync.dma_start(b2r_t[:], b2_d[:])
            if has_b1 or has_b2:
                sqd_t = const.tile([1, NPAD], F32)
                nc.sync.dma_start(sqd_t[:], sqd_d[:])
            ivd_t = const.tile([128, NT], F32)
            nc.sync.dma_start(ivd_t[:], ivd_d[:])
            bcol_t = const.tile([128, NT], F32)
            nc.sync.dma_start(bcol_t[:], bcol_d[:])
            with tc.tile_pool(name="dsw", bufs=1) as dswp:
                dstwh = dswp.tile([128, TOTCH], FP16)
                nc.sync.dma_start(dstwh[:], dstw_d[:])
                nc.any.tensor_copy(dstw_t[:], dstwh[:])
            wvc_t = const.tile([128, NT], F32)
            nc.sync.dma_start(wvc_t[:], wvc_d[:])
            # own x rows in tile layout [p, (t f)]
            NTF = NPC // 128
            TTAIL = NPC - NTF * 128
            xso_t = const.tile([128, NT * 4], FP16)
            nc.sync.dma_start(
                xso_t[:, 0:NTF * 4].rearrange("p (t f) -> p t f", f=4),
                xss_d[0:NTF * 128, :].rearrange("(t p) f -> p t f", p=128))
            if TTAIL:
                nc.sync.dma_start(xso_t[0:TTAIL, NTF * 4:NTF * 4 + 4],
                                  xss_d[NTF * 128:NPC, :])
            # resident prescaled h1 (layer-1 output, node within tile on
            # partitions, (tile, feat) on free dim)
            h1sb = const.tile([128, NPAD], FP16)

            # ---- phase 0b: build the layer-1 gather tables from the
            # replicated prescaled x. Only cols 0:4 are ever read downstream;
            # 4:128 stay garbage.
            with tc.tile_pool(name="xb", bufs=1) as xbp, \
                 tc.tile_pool(name="zs", bufs=2) as zsp:
                NB = NBF + (1 if NTAIL else 0)
                xsb_sb = xbp.tile([128, NB * 4], FP16)
                nc.sync.dma_start(
                    xsb_sb[:, 0:NBF * 4].rearrange("p (t f) -> p t f", f=4),
                    xsr_d[0:NBF * 128, :].rearrange("(t p) f -> p t f",
                                                    p=128))
                if NTAIL:
                    nc.sync.dma_start(xsb_sb[0:NTAIL, NBF * 4:NBF * 4 + 4],
                                      xsr_d[NBF * 128:N, :])
                BW = 8
                SCB = cfg.SC_SIZE // 128     # blocks per src chunk
                b0 = 0
                while b0 < NBF:
                    s = b0 // SCB
                    # keep each dma within one chunk's table
                    nb = min(BW, NBF - b0, (s + 1) * SCB - b0)
                    r0 = (b0 - s * SCB) * 128
                    zst = zsp.tile([128, BW, 128], FP16, tag="zst")
                    nc.vector.tensor_copy(
                        zst[:, 0:nb, 0:4],
                        xsb_sb[:, b0 * 4:(b0 + nb) * 4]
                        .rearrange("p (t f) -> p t f", f=4))
                    nc.sync.dma_start(
                        xpad_ts[s][r0:r0 + nb * 128, :]
                        .rearrange("(t p) f -> p t f", p=128),
                        zst[:, 0:nb, :])
                    b0 += nb
                if NTAIL:
                    s = NBF // SCB
                    r0 = (NBF - s * SCB) * 128
                    zst = zsp.tile([128, BW, 128], FP16, tag="zst")
                    nc.vector.tensor_copy(
                        zst[0:NTAIL, 0, 0:4],
                        xsb_sb[0:NTAIL, NBF * 4:NBF * 4 + 4])
                    nc.sync.dma_start(xpad_ts[s][r0:r0 + NTAIL, :],
                                      zst[0:NTAIL, 0, :])

            # ---- the two GCN layers
            def layer(lnum, tab_of, KIN, w_t, brow_t, has_bias):
                gq = 0
                CALL_CHUNKS = 48
                with ExitStack() as lctx:
                    msgp = lctx.enter_context(
                        tc.tile_pool(name=f"msg{lnum}", bufs=2))
                    ohp = lctx.enter_context(
                        tc.tile_pool(name=f"oh{lnum}", bufs=4))
                    asbp = lctx.enter_context(
                        tc.tile_pool(name=f"asb{lnum}", bufs=2))
                    rlp = lctx.enter_context(
                        tc.tile_pool(name=f"rl{lnum}", bufs=2))
                    stp = lctx.enter_context(
                        tc.tile_pool(name=f"st{lnum}", bufs=2))
                    aggp = lctx.enter_context(
                        tc.tile_pool(name=f"agg{lnum}", bufs=4, space="PSUM"))
                    p2p = lctx.enter_context(
                        tc.tile_pool(name=f"p2{lnum}", bufs=1, space="PSUM"))
                    trp = lctx.enter_context(
                        tc.tile_pool(name=f"tr{lnum}", bufs=2, space="PSUM"))
                    if lnum == 2:
                        pwp = lctx.enter_context(
                            tc.tile_pool(name="pw", bufs=3))
                        plp = lctx.enter_context(
                            tc.tile_pool(name="pl", bufs=1, space="PSUM"))
                        pooled_ps = plp.tile([128, G], F32)

                    for w, wtiles in enumerate(waves):
                        msg = msgp.tile([128, CW, 128], FP16, tag="msg")
                        for s in range(NSC):
                            icol0, nidx, mcol0, nch = gmeta[w][s]
                            if nidx == 0:
                                continue
                            for cb in range(0, nch, CALL_CHUNKS):
                                ce = min(cb + CALL_CHUNKS, nch)
                                ni = (ce - cb) * 128
                                nc.gpsimd.dma_gather(
                                    msg[:, mcol0 + cb:mcol0 + ce, :],
                                    tab_of(s),
                                    idx_t[:, icol0 + cb * 8:
                                          icol0 + cb * 8 + ni // 16],
                                    ni, ni, 128,
                                    single_packet=False,
                                    queue_num=gq % 4,
                                )
                                gq += 1
                        # zero each psum bank with a full-width PE matmul; all
                        # chunk matmuls then accumulate in any order
                        mms = wmms[w]
                        aggs = [aggp.tile([KIN, 512], F32, tag="agg",
                                          name=f"agg{lnum}_w{w}_{h}")
                                for h in range((len(wtiles) + 3) // 4)]
                        for agg in aggs:
                            nc.tensor.matmul(agg[:], zc_t[0:1, 0:KIN],
                                             zc_t[0:1, 0:512],
                                             start=True, stop=False,
                                             skip_group_check=True)
                        for mcol, j0, wid, gcol in mms:
                            oh = ohp.tile([128, wid * 128], FP16, tag="oh")
                            nc.vector.tensor_scalar(
                                oh[:], iota_t[:, j0 * 128:(j0 + wid) * 128],
                                dstw_t[:, gcol:gcol + 1], None, iseq)
                            agg = aggs[j0 // 4]
                            psl = agg[:, (j0 % 4) * 128:(j0 % 4 + wid) * 128]
                            nc.tensor.matmul(
                                psl, msg[:, mcol, 0:KIN], oh[:],
                                start=False, stop=False,
                                skip_group_check=True)
                        # self-loop rows close each bank's accumulation group
                        for j, t in enumerate(wtiles):
                            rows = min(128, NPC - t * 128)
                            if lnum == 1:
                                own_ap = xso_t[0:rows, t * 4:t * 4 + 4]
                            else:
                                own_ap = h1sb[0:rows, t * 128:t * 128 + KIN]
                            psl = aggs[j // 4][:, (j % 4) * 128:
                                               (j % 4) * 128 + 128]
                            nc.tensor.matmul(
                                psl, own_ap, identb_t[0:rows, :],
                                start=False,
                                stop=(j % 4 == 3 or j == len(wtiles) - 1),
                                skip_group_check=True)
                        for j, t in enumerate(wtiles):
                            psl = aggs[j // 4][:, (j % 4) * 128:
                                               (j % 4) * 128 + 128]
                            agg_sb = asbp.tile([KIN, 128], F32, tag="asb")
                            nc.scalar.activation(agg_sb[:], psl, copy_fn)
                            p2 = p2p.tile([128, 128], F32, tag="p2")
                            nc.tensor.matmul(p2[:], w_t[:], agg_sb[:],
                                             start=True, stop=not has_bias)
                            if has_bias:
                                nc.tensor.matmul(
                                    p2[:], brow_t[:],
                                    sqd_t[0:1, t * 128:t * 128 + 128],
                                    start=False, stop=True)
                            relu_sb = rlp.tile([128, 128], F32, tag="rl")
                            nc.scalar.activation(relu_sb[:], p2[:], relu)
                            tnm = trp.tile([128, 128], F32, tag="tr")
                            nc.tensor.transpose(tnm[:], relu_sb[:], ident_t[:])
                            if lnum == 1:
                                nc.scalar.activation(
                                    h1sb[:, t * 128:t * 128 + 128], tnm[:],
                                    copy_fn, scale=ivd_t[:, t:t + 1])
                            else:
                                tnm_sb = stp.tile([128, 128], F32, tag="tnm")
                                nc.scalar.activation(tnm_sb[:], tnm[:],
                                                     copy_fn)
                                pw_t = pwp.tile([128, G], F32, tag="pw")
                                nc.vector.tensor_scalar(
                                    pw_t[:], iotag_t[:, 0:G],
                                    bcol_t[:, t:t + 1], wvc_t[:, t:t + 1],
                                    iseq, mybir.AluOpType.mult)
                                nc.tensor.matmul(
                                    pooled_ps[:], tnm_sb[:], pw_t[:],
                                    start=(t == 0), stop=(t == NT - 1),
                                    skip_group_check=True)
                        if lnum == 1:
                            # bounce this wave's rows to DRAM for the gather
                            base = wtiles[0] * 128
                            nfull = sum(1 for t in wtiles
                                        if (t + 1) * 128 <= NPC)
                            if nfull:
                                nc.sync.dma_start(
                                    h1b_t[base:base + nfull * 128, :]
                                    .rearrange("(j p) f -> p j f", p=128),
                                    h1sb[:, base:base + nfull * 128]
                                    .rearrange("p (j f) -> p j f", f=128))
                            for t in wtiles:
                                if (t + 1) * 128 <= NPC:
                                    continue
                                rows = NPC - t * 128
                                if rows > 0:
                                    nc.sync.dma_start(
                                        h1b_t[t * 128:t * 128 + rows, :],
                                        h1sb[0:rows, t * 128:(t + 1) * 128])

                    if lnum == 2:
                        pooled_sb = const.tile([128, G], F32)
                        nc.any.tensor_copy(pooled_sb[:], pooled_ps[:])
                        outp = p2p.tile([128, 128], F32, tag="p2")
                        nc.tensor.matmul(outp[0:G, 0:OUT], pooled_sb[:],
                                         wl_t[:], start=True, stop=True,
                                         skip_group_check=True)
                        out_sb = const.tile([G, OUT], F32)
                        nc.any.tensor_copy(out_sb[:], outp[0:G, 0:OUT])
                        nc.sync.dma_start(out_d[:], out_sb[:])

            layer(1, lambda s: xpad_ts[s][:, :], 4, w1_t,
                  b1r_t if has_b1 else None, has_b1)

            nc.gpsimd.collective_compute(
                "AllGather", mybir.AluOpType.bypass, replica_groups=GROUPS,
                ins=[h1b_t[:].opt()], outs=[h1full_t[:].opt()])

            layer(2, lambda s: h1full_t[s * cfg.SC_SIZE:
                                        min(N, (s + 1) * cfg.SC_SIZE), :],
                  128, w2_t, b2r_t if has_b2 else None, has_b2)

    nc.compile()
    return nc


# --------------------------------------------------------------------------
# driver
# --------------------------------------------------------------------------

def _run(cfg, meta, W1, b1, W2, b2, Wl, bl, runner):
    NC = cfg.NCORES
    has_b1 = bool(np.any(np.asarray(b1)))
    has_b2 = bool(np.any(np.asarray(b2)))

    assert cfg.DIN <= 4
    W1p = np.zeros((4, 128), dtype=np.float32)
    W1p[:cfg.DIN] = np.asarray(W1, dtype=np.float32)

    nc = build_fused(cfg, meta, has_b1, has_b2)
    in_maps = []
    for c in range(NC):
        m = dict(
            xsr=meta["xs"], xss=meta["xss"][c],
            idx=meta["idx16"][c], dstw=meta["dstw"][c],
            w1=W1p, w2=np.asarray(W2, np.float32),
            wl=np.asarray(Wl, np.float32),
            ivd=meta["invdeg_col"][c], bcol=meta["bcol"][c],
            wvc=meta["wvc"][c],
        )
        if has_b1:
            m["b1r"] = np.asarray(b1, np.float32).reshape(1, 128)
        if has_b2:
            m["b2r"] = np.asarray(b2, np.float32).reshape(1, 128)
        if has_b1 or has_b2:
            m["sqd"] = meta["sqrtdeg_row"][c]
        in_maps.append(m)
    res = runner(nc, in_maps)
    total = np.sum([res[c]["out"] for c in range(NC)], axis=0)
    return (total + np.asarray(bl, np.float32)[None, :]).astype(np.float32)


def _hw_runner(nc, in_maps):
    core_ids = list(range(len(in_maps)))
    try:
        res = run_bass_kernel_spmd(nc, in_maps, core_ids=core_ids)
    except Exception:
        # one retry for transient tunnel/device failures
        res = run_bass_kernel_spmd(nc, in_maps, core_ids=core_ids)
    return res.results


def kernel(x, edge_index, batch, W1, b1, W2, b2, Wl, bl):
    cfg = FULL
    meta = preprocess(cfg, x, edge_index, batch)
    return _run(cfg, meta, W1, b1, W2, b2, Wl, bl, _hw_runner)
